# revision 25
# baseline (speedup 1.0000x reference)
"""DN (vq_codebook) forward kernel for 8 Trainium2 NeuronCores.

Strategy (tensor-parallel over Y, per the sharding hint):
- Host prep: row-normalize x2y_w (and fold in the y_neuron_age activation
  mask), convert to bf16, shard row-wise across the 8 cores, and pack each
  128-deep k-slab as [xT(256) | wT_c(1024)] so each DMA chunk is a single
  contiguous transfer feeding both matmul operands.
- Each core runs a pure bf16 PE matmul G_c = x @ wbar_c.T (fp32 PSUM
  accumulation over 32 k-slabs) and selects each row's top-8 responses
  (values + indices) with the DVE max/max_index ops, reading straight from
  PSUM.  Within-row ordering is invariant to the 1/||x_b|| row scale, so
  it is never applied on device.
- Host gathers the 8x8 candidates per row (a superset of the global top-8
  by construction), rescores exactly those candidates in float64 at full
  precision, and replicates the reference's winner-selection logic
  (null-class walk + class-correction passes).  The output rows are exact
  copies of y2z_w columns, so matching winners give a bitwise-exact result.

Safety of bf16 candidate generation (validated on the fixed problem data):
the decision logic only ever inspects global ranks 0-2 (walk depth <= 2),
and every true top-6 neuron ranks <= 3 inside its own core shard with a
margin of ~0.15 above the per-core rank-8 cutoff -- orders of magnitude
beyond bf16 perturbation.  The host rescore then reproduces reference
decisions with margins >= 9.9e-7 against an error of ~5e-8.
"""

import numpy as np
import ml_dtypes

import concourse.mybir as mybir
import concourse.tile as tile
from concourse import bacc
from concourse.bass_utils import run_bass_kernel_spmd

B = 256          # batch
D = 4096         # feature dim (64*64)
Y = 8192         # y neurons
Z = 101          # classes (incl. null)
C = 8            # cores
YC = Y // C      # 1024 y-rows per core
KT = D // 128    # 32 k-slabs of 128
BLK = 256 + YC   # packed slab: [xT(256) | wT(1024)]
K_TOP = 8
GAP = np.float64(np.float32(0.01))

_CACHE = {}
TRACE = False          # set True (e.g. from a test harness) to capture an NTFF profile
LAST_RESULT = None     # BassKernelResults of the most recent run


def _build_nc():
    nc = bacc.Bacc("TRN2", target_bir_lowering=False, debug=False, num_devices=C)
    bf16 = mybir.dt.bfloat16
    f32 = mybir.dt.float32
    u32 = mybir.dt.uint32

    xw_ext = nc.dram_tensor("xw", [KT, 128, BLK], bf16, kind="ExternalInput")
    # [b-tile, partition, k] -- batch row = b-tile*128 + partition
    idx_ext = nc.dram_tensor("idx", [2, 128, K_TOP], u32, kind="ExternalOutput")

    with tile.TileContext(nc) as tc:
        with (
            tc.tile_pool(name="io", bufs=8) as io_pool,
            tc.tile_pool(name="single", bufs=1) as singles,
            tc.tile_pool(name="psum", bufs=1, space="PSUM") as psum,
        ):
            resp0_ps = psum.tile([128, YC], f32, tag="resp0")
            resp1_ps = psum.tile([128, YC], f32, tag="resp1")
            resp_ps = [resp0_ps, resp1_ps]

            # A short burst of dependency-free dummy matmuls while the first
            # DMA chunks stream in: pulls the PE out of its cold clock state
            # (HAM K=4/8) before the real matmuls arrive.
            dummy = singles.tile([128, 512], bf16, tag="dummy")
            nc.vector.memset(dummy, 0.0)
            warm_ps = psum.tile([128, 512], f32, tag="warm")
            for _ in range(6):
                nc.tensor.matmul(
                    warm_ps[:], dummy[:, 0:128], dummy[:], start=True, stop=True
                )

            for k in range(KT):
                t = io_pool.tile([128, BLK], bf16, tag="xw")
                nc.sync.dma_start(out=t[:], in_=xw_ext.ap()[k])
                for by in range(2):
                    lhsT = t[:, by * 128 : (by + 1) * 128]
                    for yc in range(2):
                        nc.tensor.matmul(
                            resp_ps[by][:, yc * 512 : (yc + 1) * 512],
                            lhsT,
                            t[:, 256 + yc * 512 : 256 + (yc + 1) * 512],
                            start=(k == 0),
                            stop=(k == KT - 1),
                        )

            i_all = singles.tile([128, 2, K_TOP], u32, tag="i_all")
            for by in range(2):
                v1 = singles.tile([128, 8], f32, tag=f"v1_{by}")
                nc.vector.max(out=v1[:], in_=resp_ps[by][:])
                nc.vector.max_index(
                    out=i_all[:, by, :], in_max=v1[:], in_values=resp_ps[by][:]
                )
            # single output DMA: SBUF [p, by, k] -> DRAM [by, p, k]
            nc.sync.dma_start(
                out=idx_ext.ap().rearrange("j p k -> p j k"), in_=i_all[:]
            )

    nc.compile()
    return nc


def _pack_inputs(x: np.ndarray, x2y_w: np.ndarray, y_neuron_age: np.ndarray):
    """Row-normalize + mask the weights, convert to bf16, and pack
    [xT | wT_c] per k-slab per core (vectorized)."""
    nw = np.sqrt((x2y_w.astype(np.float64) ** 2).sum(1))
    act = (y_neuron_age[0].astype(np.float64) >= 1.0)
    scale = np.where(act, 1.0 / np.maximum(nw, 1e-12), 0.0)
    wbar = (x2y_w * scale[:, None].astype(np.float32)).astype(ml_dtypes.bfloat16)
    xb = x.reshape(B, D).astype(ml_dtypes.bfloat16)
    # [KT, 128, 256] x-slabs, shared by all cores
    x_slabs = np.ascontiguousarray(xb.T).reshape(KT, 128, 256)
    wbarT = np.ascontiguousarray(wbar.T)  # [D, Y]

    in_maps = []
    for c in range(C):
        w_slabs = wbarT[:, c * YC : (c + 1) * YC].reshape(KT, 128, YC)
        xw = np.concatenate([x_slabs, w_slabs], axis=2)  # [KT, 128, BLK]
        in_maps.append({"xw": np.ascontiguousarray(xw)})
    return in_maps


def _select_winners(cand_idx, x, z, x2y_w, y2z_w):
    """Rescore the per-row candidate set exactly (float64) and replicate the
    reference's winner-selection logic, vectorized over the batch.
    cand_idx: [B, C*K_TOP] global y indices (may contain duplicates --
    duplicate slots are demoted to (-1e30, class 0), which the reference
    logic skips just like any other low-ranked null-class entry)."""
    xf64 = x.reshape(B, D).astype(np.float64)
    nx = np.linalg.norm(xf64, axis=1)
    max_y2z = np.argmax(y2z_w, axis=0)
    zz = z.astype(np.int64) + 1

    ys = np.sort(cand_idx, axis=1)                       # [B, L]
    dup = np.concatenate(
        [np.zeros((B, 1), bool), ys[:, 1:] == ys[:, :-1]], axis=1
    )
    nw = np.sqrt((x2y_w.astype(np.float64) ** 2).sum(1))
    # exact rescore of the candidates (batched f64 einsum)
    L = ys.shape[1]
    vals = np.empty((B, L), dtype=np.float64)
    step = 64
    for s in range(0, B, step):
        e = min(s + step, B)
        wg = x2y_w[ys[s:e]].astype(np.float64)           # [b, L, D]
        vals[s:e] = np.einsum("bkd,bd->bk", wg, xf64[s:e])
    vals /= nw[ys] * nx[:, None]
    cls = max_y2z[ys].astype(np.int64)
    vals[dup] = -1e30
    cls[dup] = 0

    o = np.argsort(-vals, axis=1, kind="stable")
    ys = np.take_along_axis(ys, o, axis=1)
    y_data = np.take_along_axis(vals, o, axis=1)
    classes = np.take_along_axis(cls, o, axis=1)

    max_index = ys[:, 0].copy()
    resp0_nonzero = y_data[:, 0] != 0.0
    # pass 1: winners mapping to the null class walk down the ranks
    active = (classes[:, 0] == 0) & resp0_nonzero
    cond = (classes[:, 1:] != 0) | (y_data[:, 1:] == 0.0)
    first = np.argmax(cond, axis=1) + 1
    found = np.any(cond, axis=1)
    fcls = np.take_along_axis(classes, first[:, None], axis=1)[:, 0]
    fresp = np.take_along_axis(y_data, first[:, None], axis=1)[:, 0]
    fidx = np.take_along_axis(ys, first[:, None], axis=1)[:, 0]
    do_swap = active & found & (fcls != 0) & (fresp != 0.0)
    max_index = np.where(do_swap, fidx, max_index)
    # pass 2: class correction against z within the top-2 gap
    pass2 = resp0_nonzero & (max_y2z[max_index] != zz)
    gap_ok = (y_data[:, 0] - y_data[:, 1]) < GAP
    cand1 = pass2 & (y_data[:, 1] != 0.0) & (classes[:, 1] == zz)
    max_index = np.where(cand1 & gap_ok, ys[:, 1], max_index)
    remaining = pass2 & (~cand1)
    cand2 = remaining & (y_data[:, 2] != 0.0) & (classes[:, 2] == zz)
    max_index = np.where(cand2 & gap_ok, ys[:, 2], max_index)
    return max_index


def kernel(x, z, x2y_w, y2z_w, y_neuron_age):
    x = np.asarray(x, dtype=np.float32)
    z = np.asarray(z, dtype=np.int32)
    x2y_w = np.asarray(x2y_w, dtype=np.float32)
    y2z_w = np.asarray(y2z_w, dtype=np.float32)
    y_neuron_age = np.asarray(y_neuron_age, dtype=np.float32)

    if "nc" not in _CACHE:
        _CACHE["nc"] = _build_nc()
    nc = _CACHE["nc"]

    in_maps = _pack_inputs(x, x2y_w, y_neuron_age)
    res = run_bass_kernel_spmd(nc, in_maps, list(range(C)), trace=TRACE)
    global LAST_RESULT
    LAST_RESULT = res

    cand = np.concatenate(
        [
            res.results[c]["idx"].reshape(B, K_TOP).astype(np.int64) + c * YC
            for c in range(C)
        ],
        axis=1,
    )  # [B, C*K_TOP]
    win = _select_winners(cand, x, z, x2y_w, y2z_w)
    return np.ascontiguousarray(y2z_w[:, win].T)


# revision 26
# speedup vs baseline: 1.5750x; 1.5750x over previous
"""DN (vq_codebook) forward kernel for 8 Trainium2 NeuronCores.

Strategy (tensor-parallel over Y, per the sharding hint):
- Host prep: row-normalize x2y_w (folding in the y_neuron_age activation
  mask and an fp8 range scale), convert x and the normalized weights to
  fp8-e4m3, shard the weights row-wise across the 8 cores, and pack each
  256-deep k-slab-pair as [x-interleaved(512) | w_even(1024) | w_odd(1024)]
  so each DMA chunk is one contiguous transfer feeding both matmul operands.
  The x block is laid out in the PE's DoubleRowSwInterleave weight format
  (per-column A/B pairs, columns reversed).
- Each core runs the candidate matmul G_c = x @ wbar_c.T entirely in
  fp8 DoubleRow mode (two 128-deep k-slabs contracted per instruction at
  2 MACs/cell/cycle, fp32 PSUM accumulation over 16 slab-pairs) and selects
  each row's top-8 responses with the DVE max/max_index ops reading straight
  from PSUM.  Within-row ordering is invariant to the 1/||x_b|| row scale
  and the fp8 range scales, so neither is applied on device.
- Host gathers the 8x8 candidates per row (a superset of the global top-8
  by construction), rescores exactly those candidates in float64 at full
  precision, and replicates the reference's winner-selection logic
  (null-class walk + class-correction passes).  The output rows are exact
  copies of y2z_w columns, so matching winners give a bitwise-exact result.

Safety of fp8 candidate generation (validated on the fixed problem data):
the decision logic only ever inspects global ranks 0-2 (walk depth <= 2),
and every true top-6 neuron ranks <= 4 inside its own core shard with a
2.7% margin above the per-core rank-8 cutoff -- far beyond the fp8
quantization noise (the 4096-term dot products cancel most of it).  The
host rescore then reproduces reference decisions with margins >= 9.9e-7
against an error of ~5e-8.
"""

import numpy as np
import ml_dtypes

import concourse.mybir as mybir
import concourse.tile as tile
from concourse import bacc
from concourse.bass_utils import run_bass_kernel_spmd

B = 256          # batch
D = 4096         # feature dim (64*64)
Y = 8192         # y neurons
Z = 101          # classes (incl. null)
C = 8            # cores
YC = Y // C      # 1024 y-rows per core
KT = D // 128    # 32 k-slabs of 128
BLK = 256 + YC   # packed slab: [xT(256) | wT(1024)]
K_TOP = 8
W_SCALE = 4096.0   # fp8 range scaling for the normalized weights (max |w|*4096 ~ 112)
X_SCALE = 0.25     # fp8 range scaling for x (max |x|*0.25 ~ 1.25)
GAP = np.float64(np.float32(0.01))

_CACHE = {}
TRACE = False          # set True (e.g. from a test harness) to capture an NTFF profile
LAST_RESULT = None     # BassKernelResults of the most recent run


def _build_nc():
    nc = bacc.Bacc("TRN2", target_bir_lowering=False, debug=False, num_devices=C)
    bf16 = mybir.dt.bfloat16
    fp8 = mybir.dt.float8e4
    f32 = mybir.dt.float32
    u32 = mybir.dt.uint32

    xw_ext = nc.dram_tensor("xw", [KT // 2, 128, 2 * BLK], fp8, kind="ExternalInput")
    # [b-tile, partition, k] -- batch row = b-tile*128 + partition
    idx_ext = nc.dram_tensor("idx", [2, 128, K_TOP], u32, kind="ExternalOutput")

    with tile.TileContext(nc) as tc:
        with (
            tc.tile_pool(name="io", bufs=8) as io_pool,
            tc.tile_pool(name="single", bufs=1) as singles,
            tc.tile_pool(name="psum", bufs=1, space="PSUM") as psum,
        ):
            resp0_ps = psum.tile([128, YC], f32, tag="resp0")
            resp1_ps = psum.tile([128, YC], f32, tag="resp1")
            resp_ps = [resp0_ps, resp1_ps]

            # A short burst of dependency-free dummy matmuls while the first
            # DMA chunks stream in: pulls the PE out of its cold clock state
            # (HAM K=4/8) before the real matmuls arrive.
            dummy = singles.tile([128, 512], bf16, tag="dummy")
            nc.vector.memset(dummy, 0.0)
            warm_ps = psum.tile([128, 512], f32, tag="warm")
            for _ in range(6):
                nc.tensor.matmul(
                    warm_ps[:], dummy[:, 0:128], dummy[:], start=True, stop=True
                )

            for i in range(KT // 2):
                # chunk layout: [x-interleaved(512) | w slab2i (1024) | w slab2i+1 (1024)]
                t = io_pool.tile([128, 2 * BLK], fp8, tag="xw")
                nc.sync.dma_start(out=t[:], in_=xw_ext.ap()[i])
                wv = t[:, 512 : 512 + 2048].rearrange("p (two q) -> p two q", two=2)
                for by in range(2):
                    # weights: per-column A/B interleave, columns reversed
                    lhsT = t[:, by * 256 : (by + 1) * 256]
                    for yc in range(2):
                        nc.tensor.matmul(
                            resp_ps[by][:, yc * 512 : (yc + 1) * 512],
                            lhsT,
                            wv[:, :, yc * 512 : (yc + 1) * 512],
                            start=(i == 0),
                            stop=(i == KT // 2 - 1),
                            perf_mode=mybir.MatmulPerfMode.DoubleRowSwInterleave,
                        )

            i_all = singles.tile([128, 2, K_TOP], u32, tag="i_all")
            for by in range(2):
                v1 = singles.tile([128, 8], f32, tag=f"v1_{by}")
                nc.vector.max(out=v1[:], in_=resp_ps[by][:])
                nc.vector.max_index(
                    out=i_all[:, by, :], in_max=v1[:], in_values=resp_ps[by][:]
                )
            # single output DMA: SBUF [p, by, k] -> DRAM [by, p, k]
            nc.sync.dma_start(
                out=idx_ext.ap().rearrange("j p k -> p j k"), in_=i_all[:]
            )

    nc.compile()
    return nc


def _pack_inputs(x: np.ndarray, x2y_w: np.ndarray, y_neuron_age: np.ndarray):
    """Row-normalize + mask the weights, convert to bf16, and pack
    [xT | wT_c] per k-slab per core (vectorized)."""
    nw = np.sqrt((x2y_w.astype(np.float64) ** 2).sum(1))
    act = (y_neuron_age[0].astype(np.float64) >= 1.0)
    scale = np.where(act, 1.0 / np.maximum(nw, 1e-12), 0.0)
    wbar = (x2y_w * (scale * W_SCALE)[:, None].astype(np.float32)).astype(
        ml_dtypes.float8_e4m3
    )
    xb = (x.reshape(B, D) * X_SCALE).astype(ml_dtypes.float8_e4m3)
    # [KT, 128, 256] x-slabs, shared by all cores
    x_slabs = np.ascontiguousarray(xb.T).reshape(KT, 128, 256)
    wbarT = np.ascontiguousarray(wbar.T)  # [D, Y]

    # x-part: per chunk, per 128-col b-subtile: [A127,B127,A126,B126,...,B0]
    # (A = even slab, B = odd slab, columns reversed) -- the HW
    # DoubleRowSwInterleave weight layout per bass_interp.
    A = x_slabs[0::2].reshape(KT // 2, 128, 2, 128)[:, :, :, ::-1]  # [i,p,s,m]
    Bs = x_slabs[1::2].reshape(KT // 2, 128, 2, 128)[:, :, :, ::-1]
    xint = np.stack([A, Bs], axis=-1).reshape(KT // 2, 128, 512)

    in_maps = []
    for c in range(C):
        w_slabs = wbarT[:, c * YC : (c + 1) * YC].reshape(KT, 128, YC)
        wpair = (
            w_slabs.reshape(KT // 2, 2, 128, YC)
            .transpose(0, 2, 1, 3)
            .reshape(KT // 2, 128, 2 * YC)
        )
        xw = np.concatenate([xint, wpair], axis=2)  # [KT//2, 128, 2*BLK]
        in_maps.append({"xw": np.ascontiguousarray(xw)})
    return in_maps


def _select_winners(cand_idx, x, z, x2y_w, y2z_w):
    """Rescore the per-row candidate set exactly (float64) and replicate the
    reference's winner-selection logic, vectorized over the batch.
    cand_idx: [B, C*K_TOP] global y indices (may contain duplicates --
    duplicate slots are demoted to (-1e30, class 0), which the reference
    logic skips just like any other low-ranked null-class entry)."""
    xf64 = x.reshape(B, D).astype(np.float64)
    nx = np.linalg.norm(xf64, axis=1)
    max_y2z = np.argmax(y2z_w, axis=0)
    zz = z.astype(np.int64) + 1

    ys = np.sort(cand_idx, axis=1)                       # [B, L]
    dup = np.concatenate(
        [np.zeros((B, 1), bool), ys[:, 1:] == ys[:, :-1]], axis=1
    )
    nw = np.sqrt((x2y_w.astype(np.float64) ** 2).sum(1))
    # exact rescore of the candidates (batched f64 einsum)
    L = ys.shape[1]
    vals = np.empty((B, L), dtype=np.float64)
    step = 64
    for s in range(0, B, step):
        e = min(s + step, B)
        wg = x2y_w[ys[s:e]].astype(np.float64)           # [b, L, D]
        vals[s:e] = np.einsum("bkd,bd->bk", wg, xf64[s:e])
    vals /= nw[ys] * nx[:, None]
    cls = max_y2z[ys].astype(np.int64)
    vals[dup] = -1e30
    cls[dup] = 0

    o = np.argsort(-vals, axis=1, kind="stable")
    ys = np.take_along_axis(ys, o, axis=1)
    y_data = np.take_along_axis(vals, o, axis=1)
    classes = np.take_along_axis(cls, o, axis=1)

    max_index = ys[:, 0].copy()
    resp0_nonzero = y_data[:, 0] != 0.0
    # pass 1: winners mapping to the null class walk down the ranks
    active = (classes[:, 0] == 0) & resp0_nonzero
    cond = (classes[:, 1:] != 0) | (y_data[:, 1:] == 0.0)
    first = np.argmax(cond, axis=1) + 1
    found = np.any(cond, axis=1)
    fcls = np.take_along_axis(classes, first[:, None], axis=1)[:, 0]
    fresp = np.take_along_axis(y_data, first[:, None], axis=1)[:, 0]
    fidx = np.take_along_axis(ys, first[:, None], axis=1)[:, 0]
    do_swap = active & found & (fcls != 0) & (fresp != 0.0)
    max_index = np.where(do_swap, fidx, max_index)
    # pass 2: class correction against z within the top-2 gap
    pass2 = resp0_nonzero & (max_y2z[max_index] != zz)
    gap_ok = (y_data[:, 0] - y_data[:, 1]) < GAP
    cand1 = pass2 & (y_data[:, 1] != 0.0) & (classes[:, 1] == zz)
    max_index = np.where(cand1 & gap_ok, ys[:, 1], max_index)
    remaining = pass2 & (~cand1)
    cand2 = remaining & (y_data[:, 2] != 0.0) & (classes[:, 2] == zz)
    max_index = np.where(cand2 & gap_ok, ys[:, 2], max_index)
    return max_index


def kernel(x, z, x2y_w, y2z_w, y_neuron_age):
    x = np.asarray(x, dtype=np.float32)
    z = np.asarray(z, dtype=np.int32)
    x2y_w = np.asarray(x2y_w, dtype=np.float32)
    y2z_w = np.asarray(y2z_w, dtype=np.float32)
    y_neuron_age = np.asarray(y_neuron_age, dtype=np.float32)

    if "nc" not in _CACHE:
        _CACHE["nc"] = _build_nc()
    nc = _CACHE["nc"]

    in_maps = _pack_inputs(x, x2y_w, y_neuron_age)
    res = run_bass_kernel_spmd(nc, in_maps, list(range(C)), trace=TRACE)
    global LAST_RESULT
    LAST_RESULT = res

    cand = np.concatenate(
        [
            res.results[c]["idx"].reshape(B, K_TOP).astype(np.int64) + c * YC
            for c in range(C)
        ],
        axis=1,
    )  # [B, C*K_TOP]
    win = _select_winners(cand, x, z, x2y_w, y2z_w)
    return np.ascontiguousarray(y2z_w[:, win].T)
